# revision 1
# baseline (speedup 1.0000x reference)
"""GroupedQueryAttention kernel for 8 Trainium2 NeuronCores.

Sharding: core c = (batch b = c//2, seq-half sh = c%2). Each core computes the
full attention output for 1024 query rows of one batch: all 8 q heads
(2 kv heads), plus the q/k/v projections and the o-projection for those rows.
Host work is limited to slicing/transposing inputs and concatenating outputs.

On-device layout: scoresT [keys, queries] so softmax-exp'd probabilities feed
attn@v matmuls directly as the moving operand (no transposes anywhere).
Denominators are produced by ones-weight matmuls replicated across all 64
output partitions, so normalization is a plain elementwise multiply.
"""

import numpy as np

B, S, D = 4, 2048, 512
H, KV, DH = 8, 2, 64
SQ = S // 2  # queries per core
NCORES = 8
PAIRS = 4  # head pairs (p, p+4); p -> kv0 rows 0:64, p+4 -> kv1 rows 64:128
SCALE = 1.0 / 8.0  # 1/sqrt(DH)
PERM = [0, 4, 1, 5, 2, 6, 3, 7]  # head order: pair-major

_built = {}


def _build_nc():
    import concourse.mybir as mybir
    import concourse.tile as tile
    from concourse import bacc

    fp32 = mybir.dt.float32
    Exp = mybir.ActivationFunctionType.Exp

    nc = bacc.Bacc("TRN2", target_bir_lowering=False, debug=False,
                   num_devices=NCORES)

    xT = nc.dram_tensor("xT", [D, S], fp32, kind="ExternalInput").ap()
    wq = nc.dram_tensor("wq", [D, D], fp32, kind="ExternalInput").ap()
    wk = nc.dram_tensor("wk", [D, KV * DH], fp32, kind="ExternalInput").ap()
    wv = nc.dram_tensor("wv", [D, KV * DH], fp32, kind="ExternalInput").ap()
    wo = nc.dram_tensor("wo", [D, D], fp32, kind="ExternalInput").ap()
    bqp = nc.dram_tensor("bqp", [128, PAIRS], fp32, kind="ExternalInput").ap()
    bkvp = nc.dram_tensor("bkvp", [128, 1], fp32, kind="ExternalInput").ap()
    bvbc = nc.dram_tensor("bvbc", [128, 128], fp32, kind="ExternalInput").ap()
    bobc = nc.dram_tensor("bobc", [128, D], fp32, kind="ExternalInput").ap()
    y = nc.dram_tensor("y", [SQ, D], fp32, kind="ExternalOutput").ap()

    with tile.TileContext(nc) as tc:
        with (
            tc.tile_pool(name="consts", bufs=1) as consts,
            tc.tile_pool(name="epool", bufs=3) as epool,
            tc.tile_pool(name="opool", bufs=6) as opool,
            tc.tile_pool(name="rpool", bufs=2) as rpool,
            tc.tile_pool(name="ypool", bufs=3) as ypool,
            tc.tile_pool(name="pssc", bufs=2, space="PSUM") as pssc,
            tc.tile_pool(name="ps512", bufs=4, space="PSUM") as ps512,
        ):
            # ---- load constants / inputs ----
            xt_sb = consts.tile([128, 4, S], fp32, tag="xt")
            nc.sync.dma_start(xt_sb[:], xT.rearrange("(c p) s -> p c s", p=128))
            wq_sb = consts.tile([128, 4, D], fp32, tag="wq")
            nc.sync.dma_start(wq_sb[:], wq.rearrange("(c p) j -> p c j", p=128))
            wk_sb = consts.tile([128, 4, 128], fp32, tag="wk")
            nc.sync.dma_start(wk_sb[:], wk.rearrange("(c p) j -> p c j", p=128))
            wv_sb = consts.tile([128, 4, 128], fp32, tag="wv")
            nc.sync.dma_start(wv_sb[:], wv.rearrange("(c p) j -> p c j", p=128))
            wo_sb = consts.tile([128, 4, D], fp32, tag="wo")
            nc.sync.dma_start(wo_sb[:], wo.rearrange("(c p) j -> p c j", p=128))
            bq_sb = consts.tile([128, PAIRS], fp32, tag="bq")
            nc.sync.dma_start(bq_sb[:], bqp)
            bkv_sb = consts.tile([128, 1], fp32, tag="bkv")
            nc.sync.dma_start(bkv_sb[:], bkvp)
            bv_sb = consts.tile([128, 128], fp32, tag="bv")
            nc.sync.dma_start(bv_sb[:], bvbc)
            bo_sb = consts.tile([128, D], fp32, tag="bo")
            nc.sync.dma_start(bo_sb[:], bobc)
            ones_sb = consts.tile([128, DH], fp32, tag="ones")
            nc.vector.memset(ones_sb[:], 1.0)

            # ---- projections ----
            # kT [128 (kv0|kv1 head-dim), S]
            kt_sb = consts.tile([128, S], fp32, tag="kt")
            for sc in range(S // 512):
                ps = ps512.tile([128, 512], fp32, tag="ps512")
                for c in range(4):
                    nc.tensor.matmul(ps[:], wk_sb[:, c, :],
                                     xt_sb[:, c, sc * 512:(sc + 1) * 512],
                                     start=(c == 0), stop=(c == 3))
                nc.vector.tensor_scalar_add(kt_sb[:, sc * 512:(sc + 1) * 512],
                                            ps[:], bkv_sb[:, 0:1])
            # v natural [s-block, 128][(kv0|kv1) head-dim]
            v_sb = consts.tile([128, S // 128, 128], fp32, tag="v")
            for sb in range(S // 128):
                ps = ps512.tile([128, 512], fp32, tag="ps512")
                for c in range(4):
                    nc.tensor.matmul(ps[:, 0:128],
                                     xt_sb[:, c, sb * 128:(sb + 1) * 128],
                                     wv_sb[:, c, :],
                                     start=(c == 0), stop=(c == 3))
                nc.vector.tensor_add(v_sb[:, sb, :], ps[:, 0:128], bv_sb[:])
            # qT [128 (head p | head p+4), SQ] per pair chunk
            qt_sb = consts.tile([128, PAIRS, SQ], fp32, tag="qt")
            for pr in range(PAIRS):
                for sc in range(SQ // 512):
                    ps = ps512.tile([128, 512], fp32, tag="ps512")
                    for c in range(4):
                        nc.tensor.matmul(ps[:],
                                         wq_sb[:, c, pr * 128:(pr + 1) * 128],
                                         xt_sb[:, c, sc * 512:(sc + 1) * 512],
                                         start=(c == 0), stop=(c == 3))
                    nc.vector.tensor_scalar_add(
                        qt_sb[:, pr, sc * 512:(sc + 1) * 512], ps[:],
                        bq_sb[:, pr:pr + 1])

            # ---- attention + o-proj ----
            NKB = S // 128  # 16 key blocks
            for qc in range(SQ // 512):
                ot_tiles = []
                for pr in range(PAIRS):
                    acc = ps512.tile([128, 512], fp32, tag="ps512")
                    den = ps512.tile([128, 512], fp32, tag="ps512")
                    e_tiles = [None] * NKB

                    def attnv(kb):
                        e = e_tiles[kb]
                        nc.tensor.matmul(acc[0:64, :], v_sb[:, kb, 0:64],
                                         e[:, 0:512],
                                         start=(kb == 0), stop=(kb == NKB - 1),
                                         tile_position=(0, 0))
                        nc.tensor.matmul(acc[64:128, :], v_sb[:, kb, 64:128],
                                         e[:, 512:1024],
                                         start=(kb == 0), stop=(kb == NKB - 1),
                                         tile_position=(0, 64))
                        nc.tensor.matmul(den[0:64, :], ones_sb[:],
                                         e[:, 0:512],
                                         start=(kb == 0), stop=(kb == NKB - 1),
                                         tile_position=(0, 0))
                        nc.tensor.matmul(den[64:128, :], ones_sb[:],
                                         e[:, 512:1024],
                                         start=(kb == 0), stop=(kb == NKB - 1),
                                         tile_position=(0, 64))

                    for kb in range(NKB):
                        sc_ps = pssc.tile([128, 1024], fp32, tag="scores")
                        nc.tensor.matmul(
                            sc_ps[:, 0:512],
                            kt_sb[0:64, kb * 128:(kb + 1) * 128],
                            qt_sb[0:64, pr, qc * 512:(qc + 1) * 512])
                        nc.tensor.matmul(
                            sc_ps[:, 512:1024],
                            kt_sb[64:128, kb * 128:(kb + 1) * 128],
                            qt_sb[64:128, pr, qc * 512:(qc + 1) * 512])
                        e = epool.tile([128, 1024], fp32, tag="E")
                        e_tiles[kb] = e
                        nc.scalar.activation(e[:], sc_ps[:], Exp, scale=SCALE)
                        # software pipeline: consume previous block's probs so
                        # PE never waits on the exp of the current block
                        if kb >= 1:
                            attnv(kb - 1)
                    attnv(NKB - 1)

                    rb = rpool.tile([128, 512], fp32, tag="recip")
                    scr = rpool.tile([128, 512], fp32, tag="rscr")
                    nc.vector.reciprocal_approx_accurate(rb[:], den[:], scr[:])
                    ot = opool.tile([128, 512], fp32, tag="outT")
                    nc.vector.tensor_mul(ot[:], acc[:], rb[:])
                    ot_tiles.append(ot)
                for m in range(4):
                    yp = ps512.tile([128, 512], fp32, tag="ps512")
                    for pr2 in range(PAIRS):
                        nc.tensor.matmul(yp[:],
                                         ot_tiles[pr2][:, m * 128:(m + 1) * 128],
                                         wo_sb[:, pr2, :],
                                         start=(pr2 == 0), stop=(pr2 == 3))
                    yt = ypool.tile([128, 512], fp32, tag="y")
                    nc.vector.tensor_add(yt[:], yp[:], bo_sb[:])
                    blk = qc * 4 + m
                    nc.sync.dma_start(y[blk * 128:(blk + 1) * 128, :], yt[:])

    nc.finalize()
    return nc


def _get_nc():
    if "nc" not in _built:
        _built["nc"] = _build_nc()
    return _built["nc"]


def kernel(x, Wq, bq, Wk, bk, Wv, bv, Wo, bo):
    from concourse.bass_utils import run_bass_kernel_spmd

    x = np.ascontiguousarray(np.asarray(x, np.float32))
    Wq = np.asarray(Wq, np.float32)
    bq = np.asarray(bq, np.float32)
    Wk = np.asarray(Wk, np.float32)
    bk = np.asarray(bk, np.float32)
    Wv = np.asarray(Wv, np.float32)
    bv = np.asarray(bv, np.float32)
    Wo = np.asarray(Wo, np.float32)
    bo = np.asarray(bo, np.float32)

    wq_p = np.ascontiguousarray(
        Wq.reshape(D, H, DH)[:, PERM, :].reshape(D, D))
    wo_p = np.ascontiguousarray(Wo.reshape(H, DH, D)[PERM].reshape(D, D))
    bq_p = np.ascontiguousarray(
        bq.reshape(H, DH)[PERM].reshape(PAIRS, 128).T)
    bkv_p = np.ascontiguousarray(bk.reshape(128, 1))
    bv_bc = np.ascontiguousarray(np.tile(bv[None, :], (128, 1)))
    bo_bc = np.ascontiguousarray(np.tile(bo[None, :], (128, 1)))

    in_maps = []
    for c in range(NCORES):
        b, sh = divmod(c, 2)
        xroll = np.roll(x[b], -sh * SQ, axis=0)
        in_maps.append({
            "xT": np.ascontiguousarray(xroll.T),
            "wq": wq_p, "wk": Wk, "wv": Wv, "wo": wo_p,
            "bqp": bq_p, "bkvp": bkv_p, "bvbc": bv_bc, "bobc": bo_bc,
        })

    nc = _get_nc()
    res = run_bass_kernel_spmd(nc, in_maps, list(range(NCORES)))
    out = np.empty((B, S, D), np.float32)
    for c in range(NCORES):
        b, sh = divmod(c, 2)
        out[b, sh * SQ:(sh + 1) * SQ, :] = res.results[c]["y"]
    return out



# revision 9
# speedup vs baseline: 2.6703x; 2.6703x over previous
"""GroupedQueryAttention kernel for 8 Trainium2 NeuronCores.

Sharding: core c = (batch b = c//2, seq-half sh = c%2). Each core computes the
full attention output for 1024 query rows of one batch: all 8 q heads
(2 kv heads), plus the q/k/v projections and the o-projection for those rows.
Host work is limited to slicing/transposing/casting inputs and concatenating
outputs.

On-device layout: scoresT [keys, queries] so softmax-exp'd probabilities feed
attn@v matmuls directly as the moving operand (no transposes anywhere).

Optimizations over the fp32 baseline:
- The matmul path runs in bf16 (1 PE cycle/row vs ~4 for fp32); PSUM
  accumulation stays fp32.  x and the weights are cast on the host, the
  on-device intermediates (kT, qT, V, exp(scores), attn-out) are written as
  bf16 by their producing engues.
- Softmax denominators ride along in the attn@v matmuls: the V stationary
  carries a 65th column of ones, so row 64 of each accumulator is the
  denominator for free (eliminates the dedicated ones-matmuls, 1/3 of the
  attention PE work in the baseline).
- Denominator rows are DMA'd from PSUM partition 64 to SBUF partition 0
  (engines cannot shift partitions; DMA can), reciprocal'd there, then
  partition-broadcast on the otherwise-idle GPSIMD engine (whose ucode
  requires a partition-0 source); normalization is then a plain multiply.
- o-proj contracts per 64-row half (head p / head p+4 stationaries), which
  lets the two normalized halves live in separate partition-0-based tiles.
- x streams in four S-column chunks so projections start before the full
  tensor arrives.
"""

import numpy as np

B, S, D = 4, 2048, 512
H, KV, DH = 8, 2, 64
SQ = S // 2  # queries per core
NCORES = 8
PAIRS = 4  # head pairs (p, p+4); p -> kv0 rows 0:64, p+4 -> kv1 rows 64:128
SCALE = 1.0 / 8.0  # 1/sqrt(DH)
PERM = [0, 4, 1, 5, 2, 6, 3, 7]  # q head order: pair-major

_built = {}


def _build_nc():
    import concourse.mybir as mybir
    import concourse.tile as tile
    from concourse import bacc

    fp32 = mybir.dt.float32
    bf16 = mybir.dt.bfloat16
    Exp = mybir.ActivationFunctionType.Exp

    nc = bacc.Bacc("TRN2", target_bir_lowering=False, debug=False,
                   num_devices=NCORES)

    xT = nc.dram_tensor("xT", [D, S], bf16, kind="ExternalInput").ap()
    wq = nc.dram_tensor("wq", [D, D], bf16, kind="ExternalInput").ap()
    wk = nc.dram_tensor("wk", [D, KV * DH], bf16, kind="ExternalInput").ap()
    wv = nc.dram_tensor("wv", [D, KV * DH], bf16, kind="ExternalInput").ap()
    woa = nc.dram_tensor("woa", [DH, PAIRS * D], bf16, kind="ExternalInput").ap()
    wob = nc.dram_tensor("wob", [DH, PAIRS * D], bf16, kind="ExternalInput").ap()
    bqp = nc.dram_tensor("bqp", [128, PAIRS], fp32, kind="ExternalInput").ap()
    bkvp = nc.dram_tensor("bkvp", [128, 1], fp32, kind="ExternalInput").ap()
    bvbc = nc.dram_tensor("bvbc", [128, 128], fp32, kind="ExternalInput").ap()
    bobc = nc.dram_tensor("bobc", [128, D], fp32, kind="ExternalInput").ap()
    y = nc.dram_tensor("y", [SQ, D], fp32, kind="ExternalOutput").ap()

    NKB = S // 128  # 16 key blocks
    NSC = S // 512  # 4 column chunks of x

    with tile.TileContext(nc) as tc:
        with (
            tc.tile_pool(name="consts", bufs=1) as consts,
            tc.tile_pool(name="epool", bufs=3) as epool,
            tc.tile_pool(name="opool", bufs=10) as opool,
            tc.tile_pool(name="npool", bufs=4) as npool,
            tc.tile_pool(name="bcpool", bufs=4) as bcpool,
            tc.tile_pool(name="ypool", bufs=3) as ypool,
            tc.tile_pool(name="pssc", bufs=2, space="PSUM") as pssc,
            tc.tile_pool(name="pacc", bufs=4, space="PSUM") as pacc,
        ):
            # ---- load constants / inputs (x in S-column chunks so the
            # projections can start before the whole tensor arrives) ----
            wk_sb = consts.tile([128, 4, 128], bf16, tag="wk")
            nc.sync.dma_start(wk_sb[:], wk.rearrange("(c p) j -> p c j", p=128))
            xt_ch = []
            xr = xT.rearrange("(c p) s -> p c s", p=128)
            for sc in range(NSC):
                xch = consts.tile([128, 4, 512], bf16, tag=f"xt{sc}")
                nc.sync.dma_start(xch[:], xr[:, :, sc * 512:(sc + 1) * 512])
                xt_ch.append(xch)
            wv_sb = consts.tile([128, 4, 128], bf16, tag="wv")
            nc.sync.dma_start(wv_sb[:], wv.rearrange("(c p) j -> p c j", p=128))
            wq_sb = consts.tile([128, 4, D], bf16, tag="wq")
            nc.sync.dma_start(wq_sb[:], wq.rearrange("(c p) j -> p c j", p=128))
            woa_sb = consts.tile([DH, PAIRS, D], bf16, tag="woa")
            nc.sync.dma_start(woa_sb[:], woa.rearrange("p (k j) -> p k j", k=PAIRS))
            wob_sb = consts.tile([DH, PAIRS, D], bf16, tag="wob")
            nc.sync.dma_start(wob_sb[:], wob.rearrange("p (k j) -> p k j", k=PAIRS))
            bq_sb = consts.tile([128, PAIRS], fp32, tag="bq")
            nc.sync.dma_start(bq_sb[:], bqp)
            bkv_sb = consts.tile([128, 1], fp32, tag="bkv")
            nc.sync.dma_start(bkv_sb[:], bkvp)
            bv_sb = consts.tile([128, 128], fp32, tag="bv")
            nc.sync.dma_start(bv_sb[:], bvbc)
            bo_sb = consts.tile([128, D], fp32, tag="bo")
            nc.sync.dma_start(bo_sb[:], bobc)

            # ---- projections ----
            # kT [128 (kv0|kv1 head-dim), S]
            kt_sb = consts.tile([128, S], bf16, tag="kt")
            for sc in range(NSC):
                ps = pacc.tile([128, 512], fp32, tag="pacc")
                for c in range(4):
                    nc.tensor.matmul(ps[:], wk_sb[:, c, :],
                                     xt_ch[sc][:, c, :],
                                     start=(c == 0), stop=(c == 3))
                nc.vector.tensor_scalar_add(kt_sb[:, sc * 512:(sc + 1) * 512],
                                            ps[:], bkv_sb[:, 0:1])
            # v natural [s-block, 128] with a ones column appended per kv half:
            # cols 0:64 = v_kv0, 64 = ones, 65:129 = v_kv1, 129 = ones
            vp_sb = consts.tile([128, NKB, 130], bf16, tag="vp")
            nc.vector.memset(vp_sb[:, :, 64:65], 1.0)
            nc.vector.memset(vp_sb[:, :, 129:130], 1.0)
            for sb in range(NKB):
                ps = pacc.tile([128, 512], fp32, tag="pacc")
                xch = xt_ch[sb // 4]
                off = (sb % 4) * 128
                for c in range(4):
                    nc.tensor.matmul(ps[:, 0:128],
                                     xch[:, c, off:off + 128],
                                     wv_sb[:, c, :],
                                     start=(c == 0), stop=(c == 3))
                nc.vector.tensor_add(vp_sb[:, sb, 0:64], ps[:, 0:64],
                                     bv_sb[:, 0:64])
                nc.vector.tensor_add(vp_sb[:, sb, 65:129], ps[:, 64:128],
                                     bv_sb[:, 64:128])
            # qT [128 (head p | head p+4), SQ] per pair
            qt_sb = consts.tile([128, PAIRS, SQ], bf16, tag="qt")
            for pr in range(PAIRS):
                for sc in range(SQ // 512):
                    ps = pacc.tile([128, 512], fp32, tag="pacc")
                    for c in range(4):
                        nc.tensor.matmul(ps[:],
                                         wq_sb[:, c, pr * 128:(pr + 1) * 128],
                                         xt_ch[sc][:, c, :],
                                         start=(c == 0), stop=(c == 3))
                    nc.vector.tensor_scalar_add(
                        qt_sb[:, pr, sc * 512:(sc + 1) * 512], ps[:],
                        bq_sb[:, pr:pr + 1])

            # ---- attention + o-proj ----
            for qc in range(SQ // 512):
                ots = []
                for pr in range(PAIRS):
                    pA = pacc.tile([128, 512], fp32, tag="pacc")
                    pB = pacc.tile([128, 512], fp32, tag="pacc")
                    e_tiles = [None] * NKB

                    def attnv(kb):
                        e = e_tiles[kb]
                        nc.tensor.matmul(pA[0:65, :],
                                         vp_sb[:, kb, 0:65],
                                         e[:, 0:512],
                                         start=(kb == 0), stop=(kb == NKB - 1))
                        nc.tensor.matmul(pB[0:65, :],
                                         vp_sb[:, kb, 65:130],
                                         e[:, 512:1024],
                                         start=(kb == 0), stop=(kb == NKB - 1))

                    for kb in range(NKB):
                        sc_ps = pssc.tile([128, 1024], fp32, tag="scores")
                        nc.tensor.matmul(
                            sc_ps[:, 0:512],
                            kt_sb[0:64, kb * 128:(kb + 1) * 128],
                            qt_sb[0:64, pr, qc * 512:(qc + 1) * 512])
                        nc.tensor.matmul(
                            sc_ps[:, 512:1024],
                            kt_sb[64:128, kb * 128:(kb + 1) * 128],
                            qt_sb[64:128, pr, qc * 512:(qc + 1) * 512])
                        e = epool.tile([128, 1024], bf16, tag="E")
                        e_tiles[kb] = e
                        nc.scalar.activation(e[:], sc_ps[:], Exp, scale=SCALE)
                        # software pipeline: consume previous block's probs so
                        # PE never waits on the exp of the current block
                        if kb >= 1:
                            attnv(kb - 1)
                    attnv(NKB - 1)

                    # normalize: den_p in pA row 64, den_p+4 in pB row 64.
                    # Engines cannot shift partitions and the gpsimd broadcast
                    # ucode reads its source from partition 0 only, so copy
                    # the rows out of PSUM and DMA them down to partition 0.
                    dsb = npool.tile([65, 1024], fp32, tag="den64")
                    nc.vector.tensor_copy(dsb[64:65, 0:512], pA[64:65, :])
                    nc.vector.tensor_copy(dsb[64:65, 512:1024], pB[64:65, :])
                    d0 = npool.tile([1, 1024], fp32, tag="den0")
                    nc.sync.dma_start(d0[0:1, :], dsb[64:65, :])
                    r0 = npool.tile([1, 1024], fp32, tag="rden0")
                    s0 = npool.tile([1, 1024], fp32, tag="rscr0")
                    nc.vector.reciprocal_approx_accurate(r0[:], d0[:], s0[:])
                    rbcA = bcpool.tile([64, 512], fp32, tag="rbcA")
                    rbcB = bcpool.tile([64, 512], fp32, tag="rbcB")
                    nc.gpsimd.partition_broadcast(rbcA[:], r0[0:1, 0:512],
                                                  channels=64)
                    nc.gpsimd.partition_broadcast(rbcB[:], r0[0:1, 512:1024],
                                                  channels=64)
                    otA = opool.tile([64, 512], bf16, tag="otA")
                    otB = opool.tile([64, 512], bf16, tag="otB")
                    nc.vector.tensor_mul(otA[:], pA[0:64, :], rbcA[:])
                    nc.vector.tensor_mul(otB[:], pB[0:64, :], rbcB[:])
                    ots.append((otA, otB))
                for m in range(4):
                    yp = pacc.tile([128, 512], fp32, tag="pacc")
                    for pr2 in range(PAIRS):
                        nc.tensor.matmul(yp[:],
                                         ots[pr2][0][:, m * 128:(m + 1) * 128],
                                         woa_sb[:, pr2, :],
                                         start=(pr2 == 0), stop=False)
                        nc.tensor.matmul(yp[:],
                                         ots[pr2][1][:, m * 128:(m + 1) * 128],
                                         wob_sb[:, pr2, :],
                                         start=False, stop=(pr2 == 3))
                    yt = ypool.tile([128, 512], fp32, tag="y")
                    nc.vector.tensor_add(yt[:], yp[:], bo_sb[:])
                    blk = qc * 4 + m
                    nc.sync.dma_start(y[blk * 128:(blk + 1) * 128, :], yt[:])

    nc.finalize()
    return nc


def _get_nc():
    if "nc" not in _built:
        _built["nc"] = _build_nc()
    return _built["nc"]


def _in_maps(x, Wq, bq, Wk, bk, Wv, bv, Wo, bo):
    import ml_dtypes

    b16 = ml_dtypes.bfloat16
    x = np.ascontiguousarray(np.asarray(x, np.float32))
    Wq = np.asarray(Wq, np.float32)
    bq = np.asarray(bq, np.float32)
    Wk = np.asarray(Wk, np.float32).astype(b16)
    bk = np.asarray(bk, np.float32)
    Wv = np.asarray(Wv, np.float32).astype(b16)
    bv = np.asarray(bv, np.float32)
    Wo = np.asarray(Wo, np.float32)
    bo = np.asarray(bo, np.float32)

    wq_p = np.ascontiguousarray(
        Wq.reshape(D, H, DH)[:, PERM, :].reshape(D, D)).astype(b16)
    wo3 = Wo.reshape(H, DH, D)
    woa = np.ascontiguousarray(
        wo3[0:PAIRS].transpose(1, 0, 2).reshape(DH, PAIRS * D)).astype(b16)
    wob = np.ascontiguousarray(
        wo3[PAIRS:H].transpose(1, 0, 2).reshape(DH, PAIRS * D)).astype(b16)
    bq_p = np.ascontiguousarray(
        bq.reshape(H, DH)[PERM].reshape(PAIRS, 128).T)
    bkv_p = np.ascontiguousarray(bk.reshape(128, 1))
    bv_bc = np.ascontiguousarray(np.tile(bv[None, :], (128, 1)))
    bo_bc = np.ascontiguousarray(np.tile(bo[None, :], (128, 1)))

    in_maps = []
    for c in range(NCORES):
        b, sh = divmod(c, 2)
        xroll = np.roll(x[b], -sh * SQ, axis=0)
        in_maps.append({
            "xT": np.ascontiguousarray(xroll.T).astype(b16),
            "wq": wq_p, "wk": Wk, "wv": Wv, "woa": woa, "wob": wob,
            "bqp": bq_p, "bkvp": bkv_p, "bvbc": bv_bc, "bobc": bo_bc,
        })
    return in_maps


def kernel(x, Wq, bq, Wk, bk, Wv, bv, Wo, bo):
    from concourse.bass_utils import run_bass_kernel_spmd

    in_maps = _in_maps(x, Wq, bq, Wk, bk, Wv, bv, Wo, bo)
    nc = _get_nc()
    res = run_bass_kernel_spmd(nc, in_maps, list(range(NCORES)))
    out = np.empty((B, S, D), np.float32)
    for c in range(NCORES):
        b, sh = divmod(c, 2)
        out[b, sh * SQ:(sh + 1) * SQ, :] = res.results[c]["y"]
    return out


# revision 14
# speedup vs baseline: 2.8391x; 1.0632x over previous
"""GroupedQueryAttention kernel for 8 Trainium2 NeuronCores.

Sharding: core c = (batch b = c//2, seq-half sh = c%2). Each core computes the
full attention output for 1024 query rows of one batch: all 8 q heads
(2 kv heads), plus the q/k/v projections and the o-projection for those rows.
Host work is limited to slicing/transposing/casting inputs and concatenating
outputs.

On-device layout: scoresT [keys, queries] so softmax-exp'd probabilities feed
attn@v matmuls directly as the moving operand.

The kernel is softmax-exp bound: the Scalar engine must evaluate
H*SQ*S = 16.8M exps per core (~1.1us per [128,1024] block, 128 blocks).
Everything else is scheduled around keeping that pipeline saturated:

- The matmul path runs in bf16 (1 PE cycle/row vs ~4 for fp32); PSUM
  accumulation stays fp32.
- Softmax denominators ride along in the attn@v matmuls: the V stationary
  carries a 65th column of ones, so row 64 of each accumulator is the
  denominator for free.
- Denominator rows are DMA'd from PSUM partition 64 to SBUF partition 0
  (engines cannot shift partitions; DMA can), reciprocal'd there, then
  partition-broadcast on the otherwise-idle GPSIMD engine (whose ucode
  requires a partition-0 source); normalization is then a plain multiply.
- The attn output halves are assembled into one [128,512] tile via an
  SBUF->SBUF DMA partition shift so o-proj contracts over all 128 dims.
- x/k/v/q tensors are tiled per chunk so dependencies are fine-grained, and
  all projection + o-proj work that is not needed immediately is drained
  1-2 items per kb slot into the PE idle gaps of the exp-bound attention
  loops ("deferred work"), instead of running as serial phases.
"""

import numpy as np

B, S, D = 4, 2048, 512
H, KV, DH = 8, 2, 64
SQ = S // 2  # queries per core
NCORES = 8
PAIRS = 4  # head pairs (p, p+4); p -> kv0 rows 0:64, p+4 -> kv1 rows 64:128
SCALE = 1.0 / 8.0  # 1/sqrt(DH)
PERM = [0, 4, 1, 5, 2, 6, 3, 7]  # q head order: pair-major
NKB = S // 128  # 16 key blocks
NSC = S // 512  # 4 column chunks of x

_built = {}


def _build_nc():
    import concourse.mybir as mybir
    import concourse.tile as tile
    from concourse import bacc

    fp32 = mybir.dt.float32
    bf16 = mybir.dt.bfloat16
    Exp = mybir.ActivationFunctionType.Exp

    nc = bacc.Bacc("TRN2", target_bir_lowering=False, debug=False,
                   num_devices=NCORES)

    xT = nc.dram_tensor("xT", [D, S], bf16, kind="ExternalInput").ap()
    wq = nc.dram_tensor("wq", [D, D], bf16, kind="ExternalInput").ap()
    wk = nc.dram_tensor("wk", [D, KV * DH], bf16, kind="ExternalInput").ap()
    wv = nc.dram_tensor("wv", [D, KV * DH], bf16, kind="ExternalInput").ap()
    wo = nc.dram_tensor("wo", [D, D], bf16, kind="ExternalInput").ap()
    bqp = nc.dram_tensor("bqp", [128, PAIRS], fp32, kind="ExternalInput").ap()
    bkvp = nc.dram_tensor("bkvp", [128, 1], fp32, kind="ExternalInput").ap()
    bvbc = nc.dram_tensor("bvbc", [128, 128], fp32, kind="ExternalInput").ap()
    bobc = nc.dram_tensor("bobc", [128, D], fp32, kind="ExternalInput").ap()
    y = nc.dram_tensor("y", [SQ, D], fp32, kind="ExternalOutput").ap()

    with tile.TileContext(nc) as tc:
        with (
            tc.tile_pool(name="consts", bufs=1) as consts,
            tc.tile_pool(name="epool", bufs=3) as epool,
            tc.tile_pool(name="opool", bufs=9) as opool,
            tc.tile_pool(name="obpool", bufs=3) as obpool,
            tc.tile_pool(name="npool", bufs=4) as npool,
            tc.tile_pool(name="bcpool", bufs=4) as bcpool,
            tc.tile_pool(name="ypool", bufs=3) as ypool,
            tc.tile_pool(name="pssc", bufs=2, space="PSUM") as pssc,
            tc.tile_pool(name="pacc", bufs=4, space="PSUM") as pacc,
        ):
            # ---- input DMAs, ordered so the first projections can start
            # as early as possible ----
            wk_sb = consts.tile([128, 4, 128], bf16, tag="wk")
            nc.sync.dma_start(wk_sb[:], wk.rearrange("(c p) j -> p c j", p=128))
            xr = xT.rearrange("(c p) s -> p c s", p=128)
            xt_ch = []
            for sc in range(NSC):
                xch = consts.tile([128, 4, 512], bf16, tag=f"xt{sc}")
                xt_ch.append(xch)
            nc.sync.dma_start(xt_ch[0][:], xr[:, :, 0:512])
            wv_sb = consts.tile([128, 4, 128], bf16, tag="wv")
            nc.sync.dma_start(wv_sb[:], wv.rearrange("(c p) j -> p c j", p=128))
            nc.sync.dma_start(xt_ch[1][:], xr[:, :, 512:1024])
            wq_sb = consts.tile([128, 4, D], bf16, tag="wq")
            nc.sync.dma_start(wq_sb[:], wq.rearrange("(c p) j -> p c j", p=128))
            bq_sb = consts.tile([128, PAIRS], fp32, tag="bq")
            nc.sync.dma_start(bq_sb[:], bqp)
            bkv_sb = consts.tile([128, 1], fp32, tag="bkv")
            nc.sync.dma_start(bkv_sb[:], bkvp)
            bv_sb = consts.tile([128, 128], fp32, tag="bv")
            nc.sync.dma_start(bv_sb[:], bvbc)
            nc.sync.dma_start(xt_ch[2][:], xr[:, :, 1024:1536])
            nc.sync.dma_start(xt_ch[3][:], xr[:, :, 1536:2048])
            wo_sb = consts.tile([128, 4, D], bf16, tag="wo")
            nc.sync.dma_start(wo_sb[:], wo.rearrange("(c p) j -> p c j", p=128))
            bo_sb = consts.tile([128, D], fp32, tag="bo")
            nc.sync.dma_start(bo_sb[:], bobc)

            # per-chunk kT tiles, per-block V tiles, per-pair qT tiles so
            # consumers wait only on the piece they need
            ktt = [consts.tile([128, 512], bf16, name=f"ktt{sc}",
                               tag=f"kt{sc}") for sc in range(NSC)]
            # V block: cols 0:64 = v_kv0, 64 = ones, 65:129 = v_kv1, 129 = ones
            vpt = [consts.tile([128, 130], bf16, name=f"vpt{sb}",
                               tag=f"vp{sb}") for sb in range(NKB)]
            qtt = [consts.tile([128, SQ], bf16, name=f"qtt{pr}",
                               tag=f"qt{pr}") for pr in range(PAIRS)]

            # Projection / o-proj emitters. `ps` is the PSUM region to use:
            # in the prologue a pacc tile, inside attention jobs a 512-col
            # half of the PREVIOUS slot's scores tile (already read by its
            # exp, and the next writer is 2 slots away in PE program order).
            def kt_proj(sc, ps):
                for c in range(4):
                    nc.tensor.matmul(ps[:, 0:512], wk_sb[:, c, :],
                                     xt_ch[sc][:, c, :],
                                     start=(c == 0), stop=(c == 3))
                nc.vector.tensor_scalar_add(ktt[sc][:], ps[:, 0:512],
                                            bkv_sb[:, 0:1])

            def v_proj(sb, ps):
                xch = xt_ch[sb // 4]
                off = (sb % 4) * 128
                for c in range(4):
                    nc.tensor.matmul(ps[:, 0:128],
                                     xch[:, c, off:off + 128],
                                     wv_sb[:, c, :],
                                     start=(c == 0), stop=(c == 3))
                nc.vector.memset(vpt[sb][:, 64:65], 1.0)
                nc.vector.memset(vpt[sb][:, 129:130], 1.0)
                nc.vector.tensor_add(vpt[sb][:, 0:64], ps[:, 0:64],
                                     bv_sb[:, 0:64])
                nc.vector.tensor_add(vpt[sb][:, 65:129], ps[:, 64:128],
                                     bv_sb[:, 64:128])

            def qt_proj(pr, sc, ps):
                for c in range(4):
                    nc.tensor.matmul(ps[:, 0:512],
                                     wq_sb[:, c, pr * 128:(pr + 1) * 128],
                                     xt_ch[sc][:, c, :],
                                     start=(c == 0), stop=(c == 3))
                nc.vector.tensor_scalar_add(
                    qtt[pr][:, sc * 512:(sc + 1) * 512], ps[:, 0:512],
                    bq_sb[:, pr:pr + 1])

            ot_tiles = {}  # (qc, pr) -> assembled [128, 512] bf16 attn out

            def oproj_m(qc, m, ps):
                for pr in range(PAIRS):
                    nc.tensor.matmul(ps[:, 0:512],
                                     ot_tiles[(qc, pr)][:, m * 128:(m + 1) * 128],
                                     wo_sb[:, pr, :],
                                     start=(pr == 0), stop=(pr == 3))
                yt = ypool.tile([128, 512], fp32, tag="y")
                nc.vector.tensor_add(yt[:], ps[:, 0:512], bo_sb[:])
                blk = qc * 4 + m
                nc.sync.dma_start(y[blk * 128:(blk + 1) * 128, :], yt[:])

            def with_pacc(fn, *args):
                ps = pacc.tile([128, 512], fp32, tag="pacc")
                fn(*args, ps[:])

            # ---- serial prologue: minimum work before the exp pipeline can
            # start (kT chunk 0, V blocks 0-3, kT chunk 1, qT pair 0) ----
            with_pacc(kt_proj, 0)
            for sb in range(4):
                with_pacc(v_proj, sb)
            with_pacc(kt_proj, 1)
            with_pacc(qt_proj, 0, 0)
            with_pacc(qt_proj, 0, 1)

            # deferred work drained into the attention loops' PE idle slots:
            # {job: {slot: [closure(ps), ...]}}.  Slots 0-1 have no prev
            # scores tile in job 0, so job-0 items start at slot 2.
            def item(fn, *args):
                return lambda ps: fn(*args, ps)

            deferred = {j: {} for j in range(8)}

            def defer(j, slot, fn, *args):
                deferred[j].setdefault(slot, []).append(item(fn, *args))

            defer(0, 2, kt_proj, 2)
            defer(0, 3, kt_proj, 3)
            for sb in range(4, NKB):  # vp(k) needed by attnv(k) at slot k+1
                defer(0, sb - 1, v_proj, sb)
            defer(1, 0, qt_proj, 1, 0)
            defer(1, 2, qt_proj, 1, 1)
            defer(3, 0, qt_proj, 2, 0)
            defer(3, 2, qt_proj, 2, 1)
            defer(5, 0, qt_proj, 3, 0)
            defer(5, 2, qt_proj, 3, 1)
            # o-proj for qc0 hides in the last job (after ot(0,p3) is ready)
            defer(7, 6, oproj_m, 0, 0)
            defer(7, 9, oproj_m, 0, 1)
            defer(7, 12, oproj_m, 0, 2)
            defer(7, 15, oproj_m, 0, 3)

            # ---- 8 attention jobs: qc-major within pair so each pair's qT
            # is reused by consecutive jobs ----
            jobs = [(qc, pr) for pr in range(PAIRS) for qc in range(2)]
            prev_sc = [None]
            for j, (qc, pr) in enumerate(jobs):
                pA = pacc.tile([128, 512], fp32, tag="pacc")
                pB = pacc.tile([128, 512], fp32, tag="pacc")
                e_tiles = [None] * NKB

                def attnv(kb):
                    e = e_tiles[kb]
                    nc.tensor.matmul(pA[0:65, :], vpt[kb][:, 0:65],
                                     e[:, 0:512],
                                     start=(kb == 0), stop=(kb == NKB - 1))
                    nc.tensor.matmul(pB[0:65, :], vpt[kb][:, 65:130],
                                     e[:, 512:1024],
                                     start=(kb == 0), stop=(kb == NKB - 1))

                for kb in range(NKB):
                    sc_ps = pssc.tile([128, 1024], fp32, tag="scores")
                    nc.tensor.matmul(
                        sc_ps[:, 0:512],
                        ktt[kb // 4][0:64, (kb % 4) * 128:(kb % 4 + 1) * 128],
                        qtt[pr][0:64, qc * 512:(qc + 1) * 512])
                    nc.tensor.matmul(
                        sc_ps[:, 512:1024],
                        ktt[kb // 4][64:128, (kb % 4) * 128:(kb % 4 + 1) * 128],
                        qtt[pr][64:128, qc * 512:(qc + 1) * 512])
                    e = epool.tile([128, 1024], bf16, tag="E")
                    e_tiles[kb] = e
                    nc.scalar.activation(e[:], sc_ps[:], Exp, scale=SCALE)
                    # consume the previous block's probs so PE never waits on
                    # the exp of the current block
                    if kb >= 1:
                        attnv(kb - 1)
                    items = deferred[j].get(kb, ())
                    if items:
                        for i, fn in enumerate(items):
                            fn(prev_sc[0][:, i * 512:(i + 1) * 512])
                    prev_sc[0] = sc_ps
                attnv(NKB - 1)

                # normalize: den_p in pA row 64, den_p+4 in pB row 64.
                # Engines cannot shift partitions and the gpsimd broadcast
                # ucode reads its source from partition 0 only, so copy the
                # rows out of PSUM and DMA them down to partition 0.
                dsb = npool.tile([65, 1024], fp32, tag="den64")
                nc.vector.tensor_copy(dsb[64:65, 0:512], pA[64:65, :])
                nc.vector.tensor_copy(dsb[64:65, 512:1024], pB[64:65, :])
                d0 = npool.tile([1, 1024], fp32, tag="den0")
                nc.sync.dma_start(d0[0:1, :], dsb[64:65, :])
                r0 = npool.tile([1, 1024], fp32, tag="rden0")
                s0 = npool.tile([1, 1024], fp32, tag="rscr0")
                nc.vector.reciprocal_approx_accurate(r0[:], d0[:], s0[:])
                rbcA = bcpool.tile([64, 512], fp32, tag="rbcA")
                rbcB = bcpool.tile([64, 512], fp32, tag="rbcB")
                nc.gpsimd.partition_broadcast(rbcA[:], r0[0:1, 0:512],
                                              channels=64)
                nc.gpsimd.partition_broadcast(rbcB[:], r0[0:1, 512:1024],
                                              channels=64)
                # assemble both normalized halves into one [128, 512] tile
                # (otB via DMA partition shift) so o-proj contracts over 128
                ot = opool.tile([128, 512], bf16, tag="ot")
                nc.vector.tensor_mul(ot[0:64, :], pA[0:64, :], rbcA[:])
                obt = obpool.tile([64, 512], bf16, tag="obt")
                nc.vector.tensor_mul(obt[:], pB[0:64, :], rbcB[:])
                nc.sync.dma_start(ot[64:128, :], obt[:])
                ot_tiles[(qc, pr)] = ot

            # tail: o-proj for qc1 (qc0's was drained into job 7)
            for m in range(4):
                with_pacc(oproj_m, 1, m)

    nc.finalize()
    return nc


def _get_nc():
    if "nc" not in _built:
        _built["nc"] = _build_nc()
    return _built["nc"]


def _in_maps(x, Wq, bq, Wk, bk, Wv, bv, Wo, bo):
    import ml_dtypes

    b16 = ml_dtypes.bfloat16
    x = np.ascontiguousarray(np.asarray(x, np.float32))
    Wq = np.asarray(Wq, np.float32)
    bq = np.asarray(bq, np.float32)
    Wk = np.asarray(Wk, np.float32).astype(b16)
    bk = np.asarray(bk, np.float32)
    Wv = np.asarray(Wv, np.float32).astype(b16)
    bv = np.asarray(bv, np.float32)
    Wo = np.asarray(Wo, np.float32)
    bo = np.asarray(bo, np.float32)

    wq_p = np.ascontiguousarray(
        Wq.reshape(D, H, DH)[:, PERM, :].reshape(D, D)).astype(b16)
    wo_p = np.ascontiguousarray(
        Wo.reshape(H, DH, D)[PERM].reshape(D, D)).astype(b16)
    bq_p = np.ascontiguousarray(
        bq.reshape(H, DH)[PERM].reshape(PAIRS, 128).T)
    bkv_p = np.ascontiguousarray(bk.reshape(128, 1))
    bv_bc = np.ascontiguousarray(np.tile(bv[None, :], (128, 1)))
    bo_bc = np.ascontiguousarray(np.tile(bo[None, :], (128, 1)))

    in_maps = []
    for c in range(NCORES):
        b, sh = divmod(c, 2)
        xroll = np.roll(x[b], -sh * SQ, axis=0)
        in_maps.append({
            "xT": np.ascontiguousarray(xroll.T).astype(b16),
            "wq": wq_p, "wk": Wk, "wv": Wv, "wo": wo_p,
            "bqp": bq_p, "bkvp": bkv_p, "bvbc": bv_bc, "bobc": bo_bc,
        })
    return in_maps


def kernel(x, Wq, bq, Wk, bk, Wv, bv, Wo, bo):
    from concourse.bass_utils import run_bass_kernel_spmd

    in_maps = _in_maps(x, Wq, bq, Wk, bk, Wv, bv, Wo, bo)
    nc = _get_nc()
    res = run_bass_kernel_spmd(nc, in_maps, list(range(NCORES)))
    out = np.empty((B, S, D), np.float32)
    for c in range(NCORES):
        b, sh = divmod(c, 2)
        out[b, sh * SQ:(sh + 1) * SQ, :] = res.results[c]["y"]
    return out
